# revision 30
# baseline (speedup 1.0000x reference)
"""Multi-head attention (RoPE, causal) Trainium2 Bass kernel.

Sharding: 8 cores = DP(2 batches) x TP(4 head-quads of 4 heads each).
Each core computes, for its batch b and head block hq (heads 4hq..4hq+3):
  q/k/v projections (bf16 matmuls), RoPE on q/k, causal attention in
  "scoresT" orientation (scores[sk, sq]), and its partial slice of the
  output projection.  Host sums the 4 TP partials per batch and adds bo.

v2 changes vs v1:
  - sumexp is folded into the V weights (65-col lhsT with a ones column)
    so the dedicated ones-matmuls are gone; ctx PSUM is [65, 512] per
    head with the sumexp in row 64.
  - softmax normalization: reciprocal of the sumexp rows ([2,512] f32),
    broadcast across partitions with one tiny f32r matmul per pair
    (ones2^T @ rec2 -> [128, 512]), then one DVE mul per head.
  - causal-diagonal trimming: score/exp/ctx instructions only cover the
    allowed sq range of each sk tile; the mask multiply shrinks to the
    [128, <=512] partial region (one deduped triangle tile).

kernel(**inputs) takes the FULL unsharded inputs (numpy, keyed as in
setup_inputs) and returns the FULL [B, S, D] float32 output.
"""

import sys

if "/opt/trn_rl_repo" not in sys.path:
    sys.path.insert(0, "/opt/trn_rl_repo")

import numpy as np
import ml_dtypes

import concourse.bass as bass
import concourse.bacc as bacc
import concourse.mybir as mybir
import concourse.tile as tile
from concourse.bass_utils import run_bass_kernel_spmd

BF16 = mybir.dt.bfloat16
F32 = mybir.dt.float32
F32R = mybir.dt.float32r
NPBF16 = ml_dtypes.bfloat16

B, S, D, H, DK = 2, 2048, 1024, 16, 64
NCORES = 8
TP = 4            # head-quads per batch
HPC = H // TP     # heads per core = 4
OC = HPC * DK     # output dims per core for q/k/v projections = 256
NPAIR = HPC // 2  # head pairs per core = 2
NB = S // 512     # sq blocks of width 512
NT = S // 128     # sk tiles of width 128
ND = D // 128     # contraction d-tiles
VS = DK + 1       # per-head slot width in vsb (64 v cols + 1 ones col)

last_exec_time_ns = None
_cache = {}


def _rope_tables():
    """COS/SSIN tables [128, S]: rows j in 0:32 = cos/-sin of freq j,
    rows 32:64 = cos/+sin, repeated for the 2nd head of the pair."""
    a = np.arange(0, DK, 2, dtype=np.float32)
    inv_freq = (10000.0 ** (-2.0 * a / DK)).astype(np.float32)  # [32]
    pos = np.arange(S, dtype=np.float32)
    ang = pos[:, None] * inv_freq[None, :]          # [S, 32]
    cos = np.cos(ang).T.astype(np.float32)          # [32, S]
    sin = np.sin(ang).T.astype(np.float32)
    cos128 = np.concatenate([cos, cos, cos, cos], axis=0)     # [128, S]
    # signs baked per-row for the shifted-output t2 formulation:
    # t2[e-rows] reads ssin[o-rows] -> needs -sin; t2[o-rows] reads
    # ssin[e-rows] -> needs +sin.
    sin128 = np.concatenate([sin, -sin, sin, -sin], axis=0)   # [128, S]
    return cos128, sin128


def _analyze_mask(mask):
    """Classify [sk_tile 128] x [sq_block 512] blocks of the attention mask.

    Returns (blocks, mask_tiles):
      blocks[b] = list of (t, sq_lo, mid, mwidth) for sk tiles that are not
      fully blocked: the allowed sq range within the block is
      [sq_lo, 512); sq in [sq_lo, sq_lo+mwidth) must additionally be
      multiplied by mask_tiles[mid] ([128 sk, mwidth] bf16 0/1), the rest
      is fully allowed.  mid is None iff mwidth == 0.
    """
    m = np.asarray(mask).reshape(S, S)  # [sq, sk], nonzero = allowed
    blocks = []
    tiles = []
    keys = {}
    for b in range(NB):
        cur = []
        for t in range(NT):
            sub = (m[512 * b:512 * b + 512, 128 * t:128 * t + 128] != 0)
            if not sub.any():
                continue
            rows_any = sub.any(axis=1)
            rows_all = sub.all(axis=1)
            sq_lo = int(np.argmax(rows_any))
            if not cur and sq_lo > 0:
                # first kept tile of the block must start the full [0,512)
                # PSUM accumulation region; extend its mask with zero rows.
                sq_lo = 0
            not_all = np.nonzero(~rows_all[sq_lo:])[0]
            mwidth = 0 if len(not_all) == 0 else int(not_all[-1]) + 1
            if mwidth == 0:
                cur.append((t, sq_lo, None, 0))
            else:
                tl = np.ascontiguousarray(
                    sub[sq_lo:sq_lo + mwidth].T).astype(NPBF16)  # [128, mw]
                k = (mwidth, tl.tobytes())
                if k not in keys:
                    keys[k] = len(tiles)
                    tiles.append(tl)
                cur.append((t, sq_lo, keys[k], mwidth))
        blocks.append(cur)
    return blocks, tiles


def _build_nc(blocks, n_masks, qk_bias=False, v_bias=False, loop_n=None,
              abl=()):
    nc = bacc.Bacc(None)

    xq = nc.declare_dram_parameter("xqT", [D, S], BF16, isOutput=False)
    xk = nc.declare_dram_parameter("xkT", [D, S], BF16, isOutput=False)
    xv = nc.declare_dram_parameter("xvT", [D, S], BF16, isOutput=False)
    wq = nc.declare_dram_parameter("wqT", [D, OC], BF16, isOutput=False)
    wk = nc.declare_dram_parameter("wkT", [D, OC], BF16, isOutput=False)
    wv = nc.declare_dram_parameter("wvT", [D, OC], BF16, isOutput=False)
    wo = nc.declare_dram_parameter("woT", [OC, D], BF16, isOutput=False)
    cosd = nc.declare_dram_parameter("cos", [128, S], BF16, isOutput=False)
    ssind = nc.declare_dram_parameter("ssin", [128, S], BF16, isOutput=False)
    bqd = nc.declare_dram_parameter("bq", [128, NPAIR], F32, isOutput=False)
    bkd = nc.declare_dram_parameter("bk", [128, NPAIR], F32, isOutput=False)
    bvd = nc.declare_dram_parameter("bv", [128, OC], F32, isOutput=False)
    ones2d = nc.declare_dram_parameter("ones2", [2, 128], BF16, isOutput=False)
    nm = max(n_masks, 1)
    maskd = nc.declare_dram_parameter("masks", [nm, 128, 512], BF16,
                                      isOutput=False)
    outp = nc.declare_dram_parameter("out", [S, D], F32, isOutput=True)

    mwidths = {}
    for bl in blocks:
        for (t, lo, mid, mw) in bl:
            if mid is not None:
                mwidths[mid] = mw

    with tile.TileContext(nc) as tc:
        from contextlib import ExitStack
        with ExitStack() as ctx:
            ep = ctx.enter_context
            const = ep(tc.tile_pool(name="const", bufs=1))
            xt_p = ep(tc.tile_pool(name="xt", bufs=34))
            w_p = ep(tc.tile_pool(name="w", bufs=25))
            rope_p = ep(tc.tile_pool(name="rope", bufs=6))
            hat_p = ep(tc.tile_pool(name="hat", bufs=4))
            vsb_p = ep(tc.tile_pool(name="vsb", bufs=17))
            e_p = ep(tc.tile_pool(name="e", bufs=8))
            ctx_p = ep(tc.tile_pool(name="ctxsb", bufs=6))
            rec_p = ep(tc.tile_pool(name="rec", bufs=4))
            out_p = ep(tc.tile_pool(name="outsb", bufs=6))
            sc_ps = ep(tc.tile_pool(name="sc", bufs=2, space="PSUM"))
            ctx_ps = ep(tc.tile_pool(name="cps", bufs=2, space="PSUM"))
            acc_ps = ep(tc.tile_pool(name="acc", bufs=2, space="PSUM"))
            if loop_n is not None:
                ep(tc.For_i(0, loop_n, 1))

            # ---- constants (q/k/v weights first: they gate the PE) ----
            wts = {}
            for name, wd in (("q", wq), ("k", wk), ("v", wv)):
                lst = []
                for dt in range(ND):
                    w_t = w_p.tile([128, OC], BF16, tag="w",
                                   name=f"w_{name}{dt}")
                    nc.gpsimd.dma_start(
                        out=w_t, in_=wd[128 * dt:128 * dt + 128, :])
                    lst.append(w_t)
                wts[name] = lst
            cos_sb = const.tile([128, S], BF16)
            ssin_sb = const.tile([128, S], BF16)
            nc.gpsimd.dma_start(out=cos_sb, in_=cosd[:, :])
            nc.gpsimd.dma_start(out=ssin_sb, in_=ssind[:, :])
            wo_sb = []
            for p in range(NPAIR):
                w_t = const.tile([128, D], BF16, tag=f"wo{p}")
                nc.gpsimd.dma_start(out=w_t, in_=wo[128 * p:128 * p + 128, :])
                wo_sb.append(w_t)
            bq_sb = const.tile([128, NPAIR], F32)
            bk_sb = const.tile([128, NPAIR], F32)
            bv_sb = const.tile([128, OC], F32)
            nc.gpsimd.dma_start(out=bq_sb, in_=bqd[:, :])
            nc.gpsimd.dma_start(out=bk_sb, in_=bkd[:, :])
            nc.gpsimd.dma_start(out=bv_sb, in_=bvd[:, :])
            ones2_sb = const.tile([2, 128], BF16)
            nc.gpsimd.dma_start(out=ones2_sb, in_=ones2d[:, :])
            mask_sb = []
            for i in range(nm):
                mw = mwidths.get(i, 512)
                m_t = const.tile([128, mw], BF16, tag=f"mask{i}")
                nc.gpsimd.dma_start(out=m_t, in_=maskd[i][:, 0:mw])
                mask_sb.append(m_t)

            # ---- persistent tiles ----
            hats = {}
            for name in ("q", "k"):
                for p in range(NPAIR):
                    hats[(name, p)] = hat_p.tile(
                        [128, S], BF16, tag="hat", name=f"hat_{name}{p}")
            vsb = [vsb_p.tile([128, HPC * VS], BF16, tag="vsb",
                              name=f"vsb{st}")
                   for st in range(NT)]
            # ---- projection chunk m covers sq/sk in [512m, 512m+512) ----
            # Emitted as thunks so chunk m+1 interleaves into attention(m):
            # the PE gets proj matmuls to run while attention waits on exp.
            xds = {"q": xq, "k": xk, "v": xv}
            biases = {"q": bq_sb, "k": bk_sb}
            xts = {}

            def th_load(name, m):
                def th():
                    tiles = []
                    xd = xds[name]
                    for dt in range(ND):
                        x_t = xt_p.tile([128, 512], BF16, tag="xt")
                        nc.sync.dma_start(
                            out=x_t,
                            in_=xd[128 * dt:128 * dt + 128,
                                   512 * m:512 * m + 512])
                        tiles.append(x_t)
                    xts[(name, m)] = tiles
                return th

            def th_qk_group(name, p, m):
                def th():
                    xt = xts[(name, m)]
                    wt = wts[name]
                    raw = rope_p.tile([128, 512], BF16, tag="raw")
                    ps = acc_ps.tile([128, 512], F32, tag="acc")
                    for dt in range(ND):
                        nc.tensor.matmul(
                            ps,
                            lhsT=wt[dt][:, 128 * p:128 * p + 128],
                            rhs=xt[dt][:, :],
                            start=(dt == 0), stop=(dt == ND - 1))
                    if qk_bias:
                        tmp = rope_p.tile([128, 512], F32, tag="btmp")
                        nc.vector.tensor_copy(tmp, ps)
                        nc.vector.tensor_scalar_add(
                            raw, tmp, biases[name][:, p:p + 1])
                    else:
                        nc.vector.tensor_copy(raw, ps)
                    xts[("raw", name, p, m)] = raw
                return th

            def th_rope(name, p, m):
                def th():
                    # RoPE: hat[e] = raw[e]*cos - raw[o]*sin
                    #       hat[o] = raw[o]*cos + raw[e]*sin
                    # t2 written with partition-SHIFTED outputs (inputs stay
                    # aligned; sign baked into the ssin table rows), then a
                    # full-width add.
                    raw = xts.pop(("raw", name, p, m))
                    sl = slice(512 * m, 512 * m + 512)
                    hat = hats[(name, p)]
                    t1 = rope_p.tile([128, 512], BF16, tag="t1")
                    nc.vector.tensor_mul(t1, raw, cos_sb[:, sl])
                    t2 = rope_p.tile([128, 512], BF16, tag="t2")
                    nc.vector.tensor_mul(t2[0:32, :], raw[32:64, :],
                                         ssin_sb[32:64, sl])
                    nc.vector.tensor_mul(t2[32:64, :], raw[0:32, :],
                                         ssin_sb[0:32, sl])
                    nc.vector.tensor_mul(t2[64:96, :], raw[96:128, :],
                                         ssin_sb[96:128, sl])
                    nc.vector.tensor_mul(t2[96:128, :], raw[64:96, :],
                                         ssin_sb[64:96, sl])
                    nc.vector.tensor_add(hat[:, sl], t1, t2)
                return th

            def th_vproj(st, m):
                def th():
                    xt = xts[(("v", m))]
                    wt = wts["v"]
                    loc = st - 4 * m
                    ps = acc_ps.tile([128, 512], F32, tag="acc")
                    for dt in range(ND):
                        nc.tensor.matmul(
                            ps[:, 0:OC],
                            lhsT=xt[dt][:, 128 * loc:128 * loc + 128],
                            rhs=wt[dt][:, :],
                            start=(dt == 0), stop=(dt == ND - 1))
                    v_t = vsb[st]
                    v_view = v_t[:, 0:HPC * VS].rearrange(
                        "a (h c) -> a h c", c=VS)
                    ps_view = ps[:, 0:OC].rearrange("a (h c) -> a h c", c=DK)
                    if v_bias:
                        bv_view = bv_sb[:, :].rearrange(
                            "a (h c) -> a h c", c=DK)
                        nc.vector.tensor_add(
                            v_view[:, :, 0:DK], ps_view, bv_view)
                    else:
                        nc.vector.tensor_copy(v_view[:, :, 0:DK], ps_view)
                    nc.vector.memset(v_view[:, :, DK:DK + 1], 1.0)
                return th

            def chunk_thunks(m):
                ths = [th_load("q", m), th_load("k", m), th_load("v", m)]
                for p in range(NPAIR):
                    for name in ("q", "k"):
                        ths.append(th_qk_group(name, p, m))
                        ths.append(th_rope(name, p, m))
                for st in range(4 * m, 4 * m + 4):
                    ths.append(th_vproj(st, m))
                return ths

            # ---- attention ----
            # Software-pipelined per pair: scores/exp/mask of sk-tile i+1
            # are emitted before the ctx matmuls of sk-tile i so the PE
            # never waits on the scalar-engine exp.  One [128,1024] PSUM
            # tile per (sk-tile, pair) holds both heads side by side.
            # Pair evictions (recip -> PE broadcast -> DVE muls) are
            # deferred behind the next pair's / outproj's PE work.

            def emit_scores(b, p, t, lo, mid, mw):
                qh = hats[("q", p)]
                kh = hats[("k", p)]
                e = e_p.tile([128, 1024], BF16, tag="e")
                ps = sc_ps.tile([128, 1024], F32, tag="sc")
                nc.tensor.matmul(
                    ps[:, lo:512],
                    lhsT=kh[0:64, 128 * t:128 * t + 128],
                    rhs=qh[0:64, 512 * b + lo:512 * b + 512],
                    start=True, stop=True, tile_position=(0, 0))
                nc.tensor.matmul(
                    ps[:, 512 + lo:1024],
                    lhsT=kh[64:128, 128 * t:128 * t + 128],
                    rhs=qh[64:128, 512 * b + lo:512 * b + 512],
                    start=True, stop=True, tile_position=(64, 0))
                if lo == 0:
                    nc.scalar.activation(
                        e, ps, mybir.ActivationFunctionType.Exp)
                else:
                    ev = e[:, 0:1024].rearrange("a (h w) -> a h w", w=512)
                    pv = ps[:, 0:1024].rearrange("a (h w) -> a h w", w=512)
                    nc.scalar.activation(
                        ev[:, :, lo:512], pv[:, :, lo:512],
                        mybir.ActivationFunctionType.Exp)
                if mid is not None and "nomask" not in abl:
                    meng = nc.vector if "dvemask" in abl else nc.gpsimd
                    for h in range(2):
                        sl = slice(512 * h + lo, 512 * h + lo + mw)
                        meng.tensor_mul(e[:, sl], e[:, sl],
                                        mask_sb[mid][:, 0:mw])
                return e

            def emit_ctx(p, psA, psB, e, t, lo, first, last):
                osl = slice(lo, 512)
                if "noctxdep" in abl:
                    e = cos_sb  # break the exp->ctx dependency (perf probe)
                nc.tensor.matmul(
                    psA[:, osl],
                    lhsT=vsb[t][:, VS * 2 * p:VS * 2 * p + VS],
                    rhs=e[:, lo:512], start=first, stop=last,
                    skip_group_check=True)
                nc.tensor.matmul(
                    psB[:, osl],
                    lhsT=vsb[t][:, VS * (2 * p + 1):VS * (2 * p + 1) + VS],
                    rhs=e[:, 512 + lo:1024], start=first, stop=last,
                    skip_group_check=True)

            def make_evict(csb_un, rec2, csb):
                def pend():
                    bc = acc_ps.tile([128, 512], F32, tag="acc")
                    nc.tensor.matmul(
                        bc[0:64, 0:512],
                        lhsT=ones2_sb[0:1, 0:64],
                        rhs=rec2[0:1, 0:512],
                        start=True, stop=True)
                    nc.tensor.matmul(
                        bc[64:128, 0:512],
                        lhsT=ones2_sb[0:1, 0:64],
                        rhs=rec2[0:1, 512:1024],
                        start=True, stop=True)
                    # normalize: SBUF ctx x PSUM broadcast (one PSUM operand)
                    nc.vector.tensor_mul(csb[0:64, :], csb_un[0:64, :],
                                         bc[0:64, :])
                    nc.vector.tensor_mul(csb[64:128, :], csb_un[64:128, :],
                                         bc[64:128, :])
                return pend

            # chunk 0 is emitted up front; chunk m+1 interleaves into
            # attention(m) via the thunk queue.
            for th in chunk_thunks(0):
                th()
            pending = []
            for b in range(NB):
                if b + 1 < NB:
                    # merge next chunk's thunks with leftover (outproj)
                    # thunks: x-loads first, then alternate.
                    from itertools import zip_longest
                    ct = chunk_thunks(b + 1)
                    pending = ct[:3] + [
                        th for pair in zip_longest(ct[3:], pending)
                        for th in pair if th is not None]
                act = blocks[b]
                n = len(act)
                ctxsb = []
                for p in range(NPAIR):
                    psA = ctx_ps.tile([65, 512], F32, tag="cps")
                    psB = ctx_ps.tile([65, 512], F32, tag="cps")
                    lagq = []
                    slots = (2 - p) * n
                    for gi, (t, lo, mid, mw) in enumerate(act):
                        e = emit_scores(b, p, t, lo, mid, mw)
                        if len(lagq) >= 2:
                            emit_ctx(p, psA, psB, *lagq.pop(0))
                        lagq.append((e, t, lo, gi == 0, gi == n - 1))
                        if pending:
                            k = max(1, -(-len(pending) // max(1, slots)))
                            for _ in range(k):
                                if pending:
                                    pending.pop(0)()
                        slots -= 1
                    while lagq:
                        emit_ctx(p, psA, psB, *lagq.pop(0))
                    csb = ctx_p.tile([128, 512], BF16, tag="ctxsb")
                    if "nonorm" in abl:
                        nc.vector.tensor_copy(csb[0:64, :], psA[0:64, :])
                        nc.vector.tensor_copy(csb[64:128, :], psB[0:64, :])
                        ctxsb.append(csb)
                        continue
                    # unnormalized evict releases the cps banks immediately
                    csb_un = ctx_p.tile([128, 512], BF16, tag="ctxun")
                    nc.vector.tensor_copy(csb_un[0:64, :], psA[0:64, :])
                    nc.vector.tensor_copy(csb_un[64:128, :], psB[0:64, :])
                    rec2 = rec_p.tile([1, 1024], BF16, tag="rec")
                    # bf16 rec costs ~2^-8 relative on the normalizer
                    srcA = cos_sb[0:1, 0:512] if "recconst" in abl \
                        else psA[64:65, :]
                    srcB = cos_sb[0:1, 0:512] if "recconst" in abl \
                        else psB[64:65, :]
                    with nc.allow_low_precision(reason="bf16 reciprocal"):
                        nc.vector.reciprocal(rec2[0:1, 0:512], srcA)
                        nc.vector.reciprocal(rec2[0:1, 512:1024], srcB)
                    ctxsb.append(csb)
                    for _ in range(2):
                        if pending:
                            pending.pop(0)()
                    make_evict(csb_un, rec2, csb)()

                # outproj for block b: deferred as thunks into the next
                # block's attention windows (PE fill while exp drains).
                def th_outproj(b, j, oh, csb0, csb1):
                    def th():
                        ps = acc_ps.tile([128, 512], F32, tag="acc")
                        nc.tensor.matmul(
                            ps,
                            lhsT=csb0[:, 128 * j:128 * j + 128],
                            rhs=wo_sb[0][:, 512 * oh:512 * oh + 512],
                            start=True, stop=False, skip_group_check=True)
                        nc.tensor.matmul(
                            ps,
                            lhsT=csb1[:, 128 * j:128 * j + 128],
                            rhs=wo_sb[1][:, 512 * oh:512 * oh + 512],
                            start=False, stop=True, skip_group_check=True)
                        o_t = out_p.tile([128, 512], F32, tag="outsb")
                        if (j + oh) % 2 == 0:
                            nc.vector.tensor_copy(o_t, ps)
                        else:
                            nc.scalar.copy(o_t, ps)
                        nc.sync.dma_start(
                            out=outp[512 * b + 128 * j:
                                     512 * b + 128 * j + 128,
                                     512 * oh:512 * oh + 512],
                            in_=o_t)
                    return th

                for j in range(4):
                    for oh in range(2):
                        pending.append(
                            th_outproj(b, j, oh, ctxsb[0], ctxsb[1]))
                if b == NB - 1:
                    while pending:
                        pending.pop(0)()
    nc.finalize()
    return nc


def _prep_core_inputs(inputs, blocks, mask_tiles):
    """Build the 8 per-core input maps (host-side sharding)."""
    q = np.asarray(inputs["q"], np.float32)
    k = np.asarray(inputs["k"], np.float32)
    v = np.asarray(inputs["v"], np.float32)
    Wq = np.asarray(inputs["Wq"], np.float32)
    Wk = np.asarray(inputs["Wk"], np.float32)
    Wv = np.asarray(inputs["Wv"], np.float32)
    Wo = np.asarray(inputs["Wo"], np.float32)
    bq = np.asarray(inputs["bq"], np.float32)
    bk = np.asarray(inputs["bk"], np.float32)
    bv = np.asarray(inputs["bv"], np.float32)

    cos128, ssin128 = _rope_tables()
    cos_b = cos128.astype(NPBF16)
    ssin_b = ssin128.astype(NPBF16)
    ones2 = np.zeros((2, 128), NPBF16)
    ones2[0, 0:64] = 1.0
    ones2[1, 64:128] = 1.0
    nm = max(len(mask_tiles), 1)
    masks_t = np.zeros((nm, 128, 512), NPBF16)
    for i, t in enumerate(mask_tiles):
        masks_t[i][:, 0:t.shape[1]] = t

    # de-interleave permutation within each head: evens then odds
    perm64 = np.concatenate([np.arange(0, DK, 2), np.arange(1, DK, 2)])

    xT = {}
    for bb in range(B):
        xT[("q", bb)] = np.ascontiguousarray(q[bb].T).astype(NPBF16)
        xT[("k", bb)] = np.ascontiguousarray(k[bb].T).astype(NPBF16)
        xT[("v", bb)] = np.ascontiguousarray(v[bb].T).astype(NPBF16)

    scale = np.float32(1.0 / np.sqrt(DK))
    in_maps = []
    for c in range(NCORES):
        bb, hq = divmod(c, TP)
        rows = []
        for h in range(HPC):
            base = OC * hq + DK * h
            rows.extend((base + perm64).tolist())
        rows = np.array(rows)
        cols = np.arange(OC * hq, OC * hq + OC)

        wqT = np.ascontiguousarray(Wq[rows, :].T).astype(NPBF16)
        wkT = np.ascontiguousarray((Wk[rows, :] * scale).T).astype(NPBF16)
        wvT = np.ascontiguousarray(Wv[cols, :].T).astype(NPBF16)
        woT = np.ascontiguousarray(Wo[:, cols].T).astype(NPBF16)
        bq_t = np.ascontiguousarray(bq[rows].reshape(NPAIR, 128).T).astype(np.float32)
        bk_t = np.ascontiguousarray((bk[rows] * scale).reshape(NPAIR, 128).T).astype(np.float32)
        bv_t = np.broadcast_to(bv[cols], (128, OC)).astype(np.float32)

        in_maps.append({
            "xqT": xT[("q", bb)], "xkT": xT[("k", bb)], "xvT": xT[("v", bb)],
            "wqT": wqT, "wkT": wkT, "wvT": wvT, "woT": woT,
            "cos": cos_b, "ssin": ssin_b,
            "bq": bq_t, "bk": bk_t, "bv": bv_t,
            "ones2": ones2, "masks": masks_t,
        })
    return in_maps


def kernel(**inputs):
    global last_exec_time_ns
    import os

    mask = np.asarray(inputs["mask"])
    blocks, mask_tiles = _analyze_mask(mask)
    qk_bias = bool(np.any(np.asarray(inputs["bq"])) or np.any(np.asarray(inputs["bk"])))
    v_bias = bool(np.any(np.asarray(inputs["bv"])))
    key = (tuple(tuple(bl) for bl in blocks), len(mask_tiles), qk_bias, v_bias)
    if key not in _cache:
        _cache[key] = _build_nc(blocks, len(mask_tiles), qk_bias, v_bias)
    nc = _cache[key]

    in_maps = _prep_core_inputs(inputs, blocks, mask_tiles)
    trace = bool(os.environ.get("KERNEL_TRACE"))
    import time
    last_err = None
    for attempt in range(3):
        try:
            res = run_bass_kernel_spmd(nc, in_maps, list(range(NCORES)),
                                       trace=trace)
            break
        except Exception as e:  # transient NRT device-unrecoverable wedges
            last_err = e
            time.sleep(10.0)
    else:
        raise last_err
    last_exec_time_ns = res.exec_time_ns

    bo = np.asarray(inputs["bo"], np.float32)
    out = np.zeros((B, S, D), np.float32)
    for c in range(NCORES):
        bb = c // TP
        out[bb] += res.results[c]["out"]
    out += bo[None, None, :]
    return out
